# revision 6
# baseline (speedup 1.0000x reference)
"""Darknet 3x3 conv block (conv * mask + bias) on 8 TRN2 NeuronCores.

Problem: x[1,512,192,192] (*) w[512,512,3,3] stride1 pad1, then *mask + bias.

Strategy (v3): 1D Winograd F(4,3) along W, spatial shard over H.
  - Each core computes 24 output rows x all 512 F.
  - conv2d = sum_kh conv1d_W(x_row, w[kh]); the 1D conv uses Winograd
    F(4,3): 6 points per 4 outputs -> 2x fewer MACs than direct.
  - Host packs: x zero-padded then PHASE-SPLIT (xd[c,j,r,t] =
    xpad[c,r,4t+j]) so the device input transform reads contiguous
    slabs.  Weights pre-transformed U[p] = G @ w (exact, on host).
    Mask pre-split by output phase k.  Output left in k-split layout
    [FM,128,4,HC,T]; host unpack deinterleaves.
  - Device: input transform V = B^T d via 12 scalar_tensor_tensor /
    tensor_add ops per (cc, row-chunk), split across DVE and GpSimd by
    cc parity.  GEMM: per (fm, group of 8 rows) two PSUM tiles (points
    0-2 / 3-5, 3 banks each, pool bufs=2): 36 matmuls each, lhsT =
    U[fm,p,cc,kh] [c128,f128], rhs = V[cc,p,rows,tiles] [128,384].
    Drain: ScalarE copies PSUM->SBUF (bf16), DVE does the factored
    A^T combine (10 contiguous ops) + mask; ScalarE adds bias; one
    768KB DMA per (fm, group).
"""

import sys

for _p in ("/opt/trn_rl_repo",):
    if _p not in sys.path:
        sys.path.insert(0, _p)

import numpy as np
import ml_dtypes

N_CORES = 8
C = 512
F = 512
H = 192
W = 192
HC = H // N_CORES          # output rows per core = 24
CC = C // 128              # c chunks = 4
FM = F // 128              # f chunks = 4
P = 6                      # Winograd F(4,3) points
KH = 3                     # vertical taps (direct)
T = W // 4                 # Winograd tiles per row = 48
RG = 8                     # output rows per PSUM group
NG = HC // RG              # groups per fm = 3
NPX = RG * T               # matmul free size = 384
NWARM = 8                  # PE warmup matmuls while first DMAs land
XR = HC + 2                # x slab rows = 26

# row chunks for the input transform (independent: 1D transform).
# chunk 0 covers exactly what group g=0 needs (V rows 0..9).
CHUNKS = [(0, 10), (10, 8), (18, 8)]

_CACHE = {}


def _build():
    import concourse.bacc as bacc
    import concourse.mybir as mybir
    from concourse.tile import TileContext

    BF = mybir.dt.bfloat16
    F32 = mybir.dt.float32
    MULT = mybir.AluOpType.mult
    ADD = mybir.AluOpType.add

    nc = bacc.Bacc(trn_type="TRN2", num_devices=N_CORES)
    x_sh = nc.dram_tensor("x_sh", [128, CC, P, XR, T], BF, kind="ExternalInput")
    u_sh = nc.dram_tensor("u_sh", [128, FM, P, CC, KH, 128], BF,
                          kind="ExternalInput")
    mk_sh = nc.dram_tensor("mk_sh", [128, 4, HC, T], BF, kind="ExternalInput")
    b_sh = nc.dram_tensor("b_sh", [128, FM], F32, kind="ExternalInput")
    y_sh = nc.dram_tensor("y_sh", [FM, 128, 4, HC, T], F32,
                          kind="ExternalOutput")

    with TileContext(nc) as tc:
        with (
            tc.tile_pool(name="const", bufs=1) as cpool,
            tc.tile_pool(name="xin", bufs=4) as xpool,
            tc.tile_pool(name="vscr", bufs=2) as spool,
            tc.tile_pool(name="psum", bufs=2, space="PSUM") as ppool,
            tc.tile_pool(name="pwarm", bufs=1, space="PSUM") as wpool,
            tc.tile_pool(name="cpy", bufs=2) as cpool2,
            tc.tile_pool(name="tmp", bufs=2) as tpool,
            tc.tile_pool(name="outp", bufs=2) as opool,
        ):
            # PE warmup while the first DMAs land (HAM pre-warm + head fill)
            scratch = cpool.tile([128, NPX], BF)
            nc.vector.memset(scratch[:], 0.0)
            dps = wpool.tile([128, 512], F32, name="dps", tag="pw")
            for _ in range(NWARM):
                nc.tensor.matmul(dps[:, :NPX], scratch[:, :128], scratch[:],
                                 start=True, stop=True)

            ut = cpool.tile([128, FM, P, CC, KH, 128], BF)
            mt = cpool.tile([128, 4, HC, T], BF)
            bt = cpool.tile([128, FM], F32)
            vt = cpool.tile([128, CC, P, XR, T], BF)

            # U for fm0 (A-half points first), then mask/bias, then rest;
            # rides the ACT ring while x rides the SP ring.
            nc.scalar.dma_start(out=ut[:, 0, 0:3], in_=u_sh[:, 0, 0:3])
            nc.scalar.dma_start(out=ut[:, 0, 3:P], in_=u_sh[:, 0, 3:P])
            nc.scalar.dma_start(out=mt[:], in_=mk_sh[:])
            nc.scalar.dma_start(out=bt[:], in_=b_sh[:])
            for fm in range(1, FM):
                nc.scalar.dma_start(out=ut[:, fm], in_=u_sh[:, fm])

            # x DMAs: row-chunks, cc-interleaved, in first-use order
            xts = {}
            for ci, (r0, nr) in enumerate(CHUNKS):
                for cc in range(CC):
                    xt = xpool.tile([128, P, CHUNKS[ci][1], T], BF,
                                    name=f"x_{ci}_{cc}", tag="xt")
                    nc.sync.dma_start(out=xt[:], in_=x_sh[:, cc, :, r0:r0 + nr])
                    xts[(ci, cc)] = xt

            def transform(ci, cc):
                # GpSimd lacks TensorScalarPtr on trn2; keep DVE
                e = nc.vector
                r0, nr = CHUNKS[ci]
                xt = xts[(ci, cc)]
                st = spool.tile([128, nr, T], F32, name=f"s_{ci}_{cc}",
                                tag="st", bufs=2)
                s2 = spool.tile([128, nr, T], F32, name=f"s2_{ci}_{cc}",
                                tag="st2", bufs=2)
                v = vt[:, cc, :, r0:r0 + nr]
                d0, d1, d2 = xt[:, 0], xt[:, 1], xt[:, 2]
                d3, d4, d5 = xt[:, 3], xt[:, 4], xt[:, 5]
                # V0 = 4 d0 - 5 d2 + d4
                e.scalar_tensor_tensor(st[:], d2, -5.0, d4, MULT, ADD)
                e.scalar_tensor_tensor(v[:, 0], d0, 4.0, st[:], MULT, ADD)
                # V1 = m1 + m2, V2 = m1 - m2;  m1 = -4 d2 + d4, m2 = -4 d1 + d3
                e.scalar_tensor_tensor(st[:], d2, -4.0, d4, MULT, ADD)
                e.scalar_tensor_tensor(s2[:], d1, -4.0, d3, MULT, ADD)
                e.tensor_add(v[:, 1], st[:], s2[:])
                e.tensor_sub(v[:, 2], st[:], s2[:])
                # V3 = m3 + 2 m4, V4 = m3 - 2 m4;  m3 = d4 - d2, m4 = d3 - d1
                e.tensor_sub(st[:], d4, d2)
                e.tensor_sub(s2[:], d3, d1)
                e.scalar_tensor_tensor(v[:, 3], s2[:], 2.0, st[:], MULT, ADD)
                e.scalar_tensor_tensor(v[:, 4], s2[:], -2.0, st[:], MULT, ADD)
                # V5 = 4 d1 - 5 d3 + d5
                e.scalar_tensor_tensor(st[:], d3, -5.0, d5, MULT, ADD)
                e.scalar_tensor_tensor(v[:, 5], d1, 4.0, st[:], MULT, ADD)

            def half_mms(fm, g, pt, ph):
                # 36 accumulating matmuls for points ph*3 .. ph*3+2
                for cc in range(CC):
                    for kh in range(KH):
                        for pj in range(3):
                            p = ph * 3 + pj
                            rhs = vt[:, cc, p, RG * g + kh:RG * g + kh + RG, :]
                            nc.tensor.matmul(
                                pt[:, pj, :NPX], ut[:, fm, p, cc, kh], rhs,
                                start=(cc == 0 and kh == 0),
                                stop=(cc == CC - 1 and kh == KH - 1),
                            )

            def group(fm, g):
                ptA = ppool.tile([128, 3, 512], F32, name=f"psA_{fm}_{g}",
                                 tag="ps")
                half_mms(fm, g, ptA, 0)
                # ScalarE drains PSUM -> SBUF (bf16); DVE then runs the
                # factored A^T combine on contiguous SBUF operands.
                cp = cpool2.tile([128, 5, RG, T], BF, name=f"cp_{fm}_{g}",
                                 tag="cp")
                mA = ptA[:, :, :NPX].rearrange("p a (r t) -> p a r t", t=T)
                nc.scalar.copy(cp[:, 0:3], mA)

                ptB = ppool.tile([128, 3, 512], F32, name=f"psB_{fm}_{g}",
                                 tag="ps")
                half_mms(fm, g, ptB, 1)
                mB = ptB[:, 0:2, :NPX].rearrange("p a (r t) -> p a r t", t=T)
                nc.scalar.copy(cp[:, 3:5], mB)

                m0, m1, m2 = cp[:, 0], cp[:, 1], cp[:, 2]
                m3, m4 = cp[:, 3], cp[:, 4]
                m5 = ptB[:, 2, :NPX].rearrange("p (r t) -> p r t", t=T)
                tmp = tpool.tile([128, 6, RG, T], F32, name=f"tm_{fm}_{g}",
                                 tag="tm")
                s, dd, t0 = tmp[:, 0], tmp[:, 1], tmp[:, 2]
                pp, q, y3b = tmp[:, 3], tmp[:, 4], tmp[:, 5]
                nc.vector.tensor_add(s, m1, m2)
                nc.vector.tensor_sub(dd, m1, m2)
                nc.vector.tensor_add(t0, m0, s)
                nc.vector.tensor_add(pp, m3, m4)
                nc.vector.tensor_sub(q, m3, m4)
                nc.vector.scalar_tensor_tensor(y3b, q, 8.0, m5, MULT, ADD)

                yt = opool.tile([128, 4, RG, T], F32, name=f"y_{fm}_{g}",
                                tag="yt")
                nc.vector.tensor_add(yt[:, 0], t0, pp)
                nc.vector.scalar_tensor_tensor(yt[:, 1], q, 2.0, dd, MULT, ADD)
                nc.vector.scalar_tensor_tensor(yt[:, 2], pp, 4.0, s, MULT, ADD)
                nc.vector.tensor_add(yt[:, 3], y3b, dd)

                nc.vector.tensor_mul(yt[:], yt[:],
                                     mt[:, :, RG * g:RG * (g + 1)])
                nc.scalar.activation(
                    yt[:], yt[:],
                    mybir.ActivationFunctionType.Identity,
                    bias=bt[:, fm:fm + 1],
                )
                nc.sync.dma_start(out=y_sh[fm, :, :, RG * g:RG * (g + 1)],
                                  in_=yt[:])

            # interleave emission: chunk transforms feeding early groups
            # first, drains in between so PSUM recycles promptly.
            for cc in range(CC):
                transform(0, cc)
            for cc in range(CC):
                transform(1, cc)
            group(0, 0)
            for cc in range(CC):
                transform(2, cc)
            group(0, 1)
            group(0, 2)
            for fm in range(1, FM):
                for g in range(NG):
                    group(fm, g)

    nc.compile()
    return nc


def _wino_mats():
    BT = np.array([
        [4, 0, -5, 0, 1, 0],
        [0, -4, -4, 1, 1, 0],
        [0, 4, -4, -1, 1, 0],
        [0, -2, -1, 2, 1, 0],
        [0, 2, -1, -2, 1, 0],
        [0, 4, 0, -5, 0, 1]], dtype=np.float64)
    G = np.array([
        [1 / 4, 0, 0],
        [-1 / 6, -1 / 6, -1 / 6],
        [-1 / 6, 1 / 6, -1 / 6],
        [1 / 24, 1 / 12, 1 / 6],
        [1 / 24, -1 / 12, 1 / 6],
        [0, 0, 1]], dtype=np.float64)
    AT = np.array([
        [1, 1, 1, 1, 1, 0],
        [0, 1, -1, 2, -2, 0],
        [0, 1, 1, 4, 4, 0],
        [0, 1, -1, 8, -8, 1]], dtype=np.float64)
    return BT, G, AT


def _pack(x, w, b, mask):
    x = np.asarray(x, dtype=np.float32)
    w = np.asarray(w, dtype=np.float32)
    b = np.asarray(b, dtype=np.float32)
    mask = np.asarray(mask)

    xp = np.zeros((C, H + 2, W + 2), dtype=np.float32)
    xp[:, 1:-1, 1:-1] = x[0]
    # phase split: xd[c, j, r, t] = xp[c, r, 4t + j],  j = 0..5, t = 0..47
    xd = np.empty((C, P, H + 2, T), dtype=np.float32)
    for j in range(P):
        xd[:, j] = xp[:, :, j:j + 4 * (T - 1) + 1:4]
    xd = xd.astype(ml_dtypes.bfloat16)

    _, G, _ = _wino_mats()
    # U[p, f, c, kh] = sum_j G[p, j] * w[f, c, kh, j]
    u = np.einsum("pj,fckj->pfck", G, w.astype(np.float64)).astype(np.float32)
    # -> [c_local, fm, p, cc, kh, f_local]
    u = u.reshape(P, FM, 128, CC, 128, KH)
    u = u.transpose(4, 1, 0, 3, 5, 2)
    u = np.ascontiguousarray(u).astype(ml_dtypes.bfloat16)

    b_re = np.ascontiguousarray(b.reshape(FM, 128).T)

    # mask k-split: mk[k, h, t] = mask[h, 4t + k]
    mk = np.ascontiguousarray(
        mask.reshape(H, T, 4).transpose(2, 0, 1)).astype(ml_dtypes.bfloat16)
    in_maps = []
    for k in range(N_CORES):
        xs = xd[:, :, HC * k:HC * k + XR, :]                # [512, 6, 26, 48]
        xs = np.ascontiguousarray(
            xs.reshape(CC, 128, P, XR, T).transpose(1, 0, 2, 3, 4))
        ms = mk[:, HC * k:HC * k + HC][None]                # [1, 4, 24, 48]
        in_maps.append({
            "x_sh": xs,
            "u_sh": u,
            "mk_sh": np.ascontiguousarray(
                np.broadcast_to(ms, (128, 4, HC, T))),
            "b_sh": b_re,
        })
    return in_maps


def _unpack(results):
    slabs = []
    for k in range(N_CORES):
        ys = results[k]["y_sh"]                    # [FM, 128, 4, HC, T]
        ys = ys.reshape(F, 4, HC, T).transpose(0, 2, 3, 1)  # [F, HC, T, 4]
        slabs.append(ys.reshape(F, HC, W))
    out = np.concatenate(slabs, axis=1)                     # [512, 192, 192]
    return out[None].astype(np.float32)


def _run(inputs, **run_kwargs):
    from concourse.bass_utils import run_bass_kernel_spmd

    if "nc" not in _CACHE:
        _CACHE["nc"] = _build()
    nc = _CACHE["nc"]
    in_maps = _pack(inputs["x"], inputs["w"], inputs["b"], inputs["mask"])
    res = run_bass_kernel_spmd(nc, in_maps, core_ids=list(range(N_CORES)), **run_kwargs)
    return _unpack(res.results), res


def kernel(**inputs):
    out, _ = _run(inputs)
    return out


# revision 7
# speedup vs baseline: 1.1131x; 1.1131x over previous
"""Darknet 3x3 conv block (conv * mask + bias) on 8 TRN2 NeuronCores.

Problem: x[1,512,192,192] (*) w[512,512,3,3] stride1 pad1, then *mask + bias.

Strategy (v4): 1D Winograd F(4,3) along W, spatial shard over H.
  - Each core computes 24 output rows x all 512 F.
  - conv2d = sum_kh conv1d_W(x_row, w[kh]); the 1D conv uses Winograd
    F(4,3): 6 points per 4 outputs -> 2x fewer MACs than direct.
  - Host packs: x zero-padded, phase-split (xd[c,j,r,t] = xpad[c,r,4t+j])
    and laid out as flat per-(chunk, cc-pair) segments so every x DMA is
    one long contiguous run per partition.  Weights pre-transformed
    U[p] = G @ w (exact, on host).  Mask pre-split by output phase k.
    Output bf16 in per-(fm,g) contiguous segments; host deinterleaves
    and upcasts.
  - Device: input transform V = B^T d via 12 bf16 scalar_tensor_tensor /
    tensor_add ops per (row-chunk, cc-pair) on DVE.  GEMM: per (fm,
    group of 8 rows) two PSUM tiles (points 0-2 / 3-5, 3 banks each,
    pool bufs=2): 36 matmuls each, lhsT = U[fm,p,cc,kh] [c128,f128],
    rhs = V[cc,p,rows,tiles] [128,384].  Drain: ScalarE copies all of
    PSUM -> SBUF bf16 (frees PSUM without touching DVE), DVE does the
    factored A^T combine + mask, ScalarE adds bias, one bf16 DMA per
    (fm, group).
"""

import sys

for _p in ("/opt/trn_rl_repo",):
    if _p not in sys.path:
        sys.path.insert(0, _p)

import numpy as np
import ml_dtypes

N_CORES = 8
C = 512
F = 512
H = 192
W = 192
HC = H // N_CORES          # output rows per core = 24
CC = C // 128              # c chunks = 4
FM = F // 128              # f chunks = 4
P = 6                      # Winograd F(4,3) points
KH = 3                     # vertical taps (direct)
T = W // 4                 # Winograd tiles per row = 48
RG = 8                     # output rows per PSUM group
NG = HC // RG              # groups per fm = 3
NPX = RG * T               # matmul free size = 384
NWARM = 8                  # PE warmup matmuls while first DMAs land
XR = HC + 2                # x slab rows = 26

# row chunks for the input transform (independent: 1D transform).
# chunk 0 covers exactly what group g=0 needs (V rows 0..9).
CHUNKS = [(0, 10), (10, 8), (18, 8)]
# flat x segment layout: per (chunk, cc-pair) a contiguous run of
# 2 * P * nr * T bf16 elements per partition.
XSEG = [2 * P * nr * T for _, nr in CHUNKS]
XOFF = {}
_off = 0
for _ci in range(len(CHUNKS)):
    for _pr in range(2):
        XOFF[(_ci, _pr)] = _off
        _off += XSEG[_ci]
XTOT = _off

_CACHE = {}


def _build():
    import concourse.bacc as bacc
    import concourse.mybir as mybir
    from concourse.tile import TileContext

    BF = mybir.dt.bfloat16
    F32 = mybir.dt.float32
    MULT = mybir.AluOpType.mult
    ADD = mybir.AluOpType.add

    nc = bacc.Bacc(trn_type="TRN2", num_devices=N_CORES)
    x_sh = nc.dram_tensor("x_sh", [128, XTOT], BF, kind="ExternalInput")
    u_sh = nc.dram_tensor("u_sh", [128, FM, P, CC, KH, 128], BF,
                          kind="ExternalInput")
    mk_sh = nc.dram_tensor("mk_sh", [128, 4, HC, T], BF, kind="ExternalInput")
    b_sh = nc.dram_tensor("b_sh", [128, FM], F32, kind="ExternalInput")
    y_sh = nc.dram_tensor("y_sh", [FM, NG, 128, 4, RG, T], BF,
                          kind="ExternalOutput")

    with TileContext(nc) as tc:
        with (
            tc.tile_pool(name="const", bufs=1) as cpool,
            tc.tile_pool(name="xin", bufs=2) as xpool,
            tc.tile_pool(name="vscr", bufs=2) as spool,
            tc.tile_pool(name="psum", bufs=2, space="PSUM") as ppool,
            tc.tile_pool(name="pwarm", bufs=1, space="PSUM") as wpool,
            tc.tile_pool(name="cpy", bufs=2) as cpool2,
            tc.tile_pool(name="tmp", bufs=2) as tpool,
            tc.tile_pool(name="outp", bufs=2) as opool,
        ):
            # PE warmup while the first DMAs land (HAM pre-warm + head fill)
            scratch = cpool.tile([128, NPX], BF)
            nc.vector.memset(scratch[:], 0.0)
            dps = wpool.tile([128, 512], F32, name="dps", tag="pw")
            for _ in range(NWARM):
                nc.tensor.matmul(dps[:, :NPX], scratch[:, :128], scratch[:],
                                 start=True, stop=True)

            ut = cpool.tile([128, FM, P, CC, KH, 128], BF)
            mt = cpool.tile([128, 4, HC, T], BF)
            bt = cpool.tile([128, FM], F32)
            vt = cpool.tile([128, CC, P, XR, T], BF)

            # U for fm0 (A-half points first), then mask/bias, then rest;
            # rides the ACT ring while x rides the SP ring.
            nc.scalar.dma_start(out=ut[:, 0, 0:3], in_=u_sh[:, 0, 0:3])
            nc.scalar.dma_start(out=ut[:, 0, 3:P], in_=u_sh[:, 0, 3:P])
            nc.scalar.dma_start(out=mt[:], in_=mk_sh[:])
            nc.scalar.dma_start(out=bt[:], in_=b_sh[:])
            for fm in range(1, FM):
                nc.scalar.dma_start(out=ut[:, fm], in_=u_sh[:, fm])

            # x DMAs: per (row-chunk, cc-pair), contiguous flat segments,
            # in first-use order
            xts = {}
            for ci, (r0, nr) in enumerate(CHUNKS):
                for pr in range(2):
                    xt = xpool.tile([128, 2, P, nr, T], BF,
                                    name=f"x_{ci}_{pr}", tag="xt")
                    seg = x_sh[:, XOFF[(ci, pr)]:XOFF[(ci, pr)] + XSEG[ci]]
                    nc.sync.dma_start(
                        out=xt[:],
                        in_=seg.rearrange("p (c j r t) -> p c j r t",
                                          c=2, j=P, t=T))
                    xts[(ci, pr)] = xt

            def transform(ci, pr):
                r0, nr = CHUNKS[ci]
                xt = xts[(ci, pr)]
                st = spool.tile([128, 2, nr, T], BF, name=f"s_{ci}_{pr}",
                                tag="st", bufs=2)
                s2 = spool.tile([128, 2, nr, T], BF, name=f"s2_{ci}_{pr}",
                                tag="st2", bufs=2)
                # v slice covers the cc pair: [2, nr, T] per point
                v = vt[:, 2 * pr:2 * pr + 2, :, r0:r0 + nr]
                d0, d1, d2 = xt[:, :, 0], xt[:, :, 1], xt[:, :, 2]
                d3, d4, d5 = xt[:, :, 3], xt[:, :, 4], xt[:, :, 5]
                e = nc.vector
                # A-half points first so fm0-g0 A matmuls start early.
                # V0 = 4 d0 - 5 d2 + d4
                e.scalar_tensor_tensor(st[:], d2, -5.0, d4, MULT, ADD)
                e.scalar_tensor_tensor(v[:, :, 0], d0, 4.0, st[:], MULT, ADD)
                # V1 = m1 + m2, V2 = m1 - m2;  m1 = -4 d2 + d4, m2 = -4 d1 + d3
                e.scalar_tensor_tensor(st[:], d2, -4.0, d4, MULT, ADD)
                e.scalar_tensor_tensor(s2[:], d1, -4.0, d3, MULT, ADD)
                e.tensor_add(v[:, :, 1], st[:], s2[:])
                e.tensor_sub(v[:, :, 2], st[:], s2[:])
                # V3 = m3 + 2 m4, V4 = m3 - 2 m4;  m3 = d4 - d2, m4 = d3 - d1
                e.tensor_sub(st[:], d4, d2)
                e.tensor_sub(s2[:], d3, d1)
                e.scalar_tensor_tensor(v[:, :, 3], s2[:], 2.0, st[:], MULT, ADD)
                e.scalar_tensor_tensor(v[:, :, 4], s2[:], -2.0, st[:], MULT, ADD)
                # V5 = 4 d1 - 5 d3 + d5
                e.scalar_tensor_tensor(st[:], d3, -5.0, d5, MULT, ADD)
                e.scalar_tensor_tensor(v[:, :, 5], d1, 4.0, st[:], MULT, ADD)

            def half_mms(fm, g, pt, ph):
                # 36 accumulating matmuls for points ph*3 .. ph*3+2
                for cc in range(CC):
                    for kh in range(KH):
                        for pj in range(3):
                            p = ph * 3 + pj
                            rhs = vt[:, cc, p, RG * g + kh:RG * g + kh + RG, :]
                            nc.tensor.matmul(
                                pt[:, pj, :NPX], ut[:, fm, p, cc, kh], rhs,
                                start=(cc == 0 and kh == 0),
                                stop=(cc == CC - 1 and kh == KH - 1),
                            )

            def group(fm, g):
                ptA = ppool.tile([128, 3, 512], F32, name=f"psA_{fm}_{g}",
                                 tag="ps")
                half_mms(fm, g, ptA, 0)
                # ScalarE drains all of PSUM -> SBUF (bf16): frees both
                # PSUM tiles without touching the DVE queue.
                cp = cpool2.tile([128, 6, RG, T], BF, name=f"cp_{fm}_{g}",
                                 tag="cp")
                mA = ptA[:, :, :NPX].rearrange("p a (r t) -> p a r t", t=T)
                nc.scalar.copy(cp[:, 0:3], mA)

                ptB = ppool.tile([128, 3, 512], F32, name=f"psB_{fm}_{g}",
                                 tag="ps")
                half_mms(fm, g, ptB, 1)
                mB = ptB[:, :, :NPX].rearrange("p a (r t) -> p a r t", t=T)
                nc.scalar.copy(cp[:, 3:6], mB)

                m0, m1, m2 = cp[:, 0], cp[:, 1], cp[:, 2]
                m3, m4, m5 = cp[:, 3], cp[:, 4], cp[:, 5]
                tmp = tpool.tile([128, 6, RG, T], BF, name=f"tm_{fm}_{g}",
                                 tag="tm")
                s, dd, t0 = tmp[:, 0], tmp[:, 1], tmp[:, 2]
                pp, q, y3b = tmp[:, 3], tmp[:, 4], tmp[:, 5]
                nc.vector.tensor_add(s, m1, m2)
                nc.vector.tensor_sub(dd, m1, m2)
                nc.vector.tensor_add(t0, m0, s)
                nc.vector.tensor_add(pp, m3, m4)
                nc.vector.tensor_sub(q, m3, m4)
                nc.vector.scalar_tensor_tensor(y3b, q, 8.0, m5, MULT, ADD)

                yt = opool.tile([128, 4, RG, T], BF, name=f"y_{fm}_{g}",
                                tag="yt")
                nc.vector.tensor_add(yt[:, 0], t0, pp)
                nc.vector.scalar_tensor_tensor(yt[:, 1], q, 2.0, dd, MULT, ADD)
                nc.vector.scalar_tensor_tensor(yt[:, 2], pp, 4.0, s, MULT, ADD)
                nc.vector.tensor_add(yt[:, 3], y3b, dd)

                nc.vector.tensor_mul(yt[:], yt[:],
                                     mt[:, :, RG * g:RG * (g + 1)])
                nc.scalar.activation(
                    yt[:], yt[:],
                    mybir.ActivationFunctionType.Identity,
                    bias=bt[:, fm:fm + 1],
                )
                nc.sync.dma_start(out=y_sh[fm, g], in_=yt[:])

            # interleave emission: chunk transforms feeding early groups
            # first, drains in between so PSUM recycles promptly.
            for pr in range(2):
                transform(0, pr)
            for pr in range(2):
                transform(1, pr)
            group(0, 0)
            for pr in range(2):
                transform(2, pr)
            group(0, 1)
            group(0, 2)
            for fm in range(1, FM):
                for g in range(NG):
                    group(fm, g)

    nc.compile()
    return nc


def _wino_mats():
    BT = np.array([
        [4, 0, -5, 0, 1, 0],
        [0, -4, -4, 1, 1, 0],
        [0, 4, -4, -1, 1, 0],
        [0, -2, -1, 2, 1, 0],
        [0, 2, -1, -2, 1, 0],
        [0, 4, 0, -5, 0, 1]], dtype=np.float64)
    G = np.array([
        [1 / 4, 0, 0],
        [-1 / 6, -1 / 6, -1 / 6],
        [-1 / 6, 1 / 6, -1 / 6],
        [1 / 24, 1 / 12, 1 / 6],
        [1 / 24, -1 / 12, 1 / 6],
        [0, 0, 1]], dtype=np.float64)
    AT = np.array([
        [1, 1, 1, 1, 1, 0],
        [0, 1, -1, 2, -2, 0],
        [0, 1, 1, 4, 4, 0],
        [0, 1, -1, 8, -8, 1]], dtype=np.float64)
    return BT, G, AT


def _pack(x, w, b, mask):
    x = np.asarray(x, dtype=np.float32)
    w = np.asarray(w, dtype=np.float32)
    b = np.asarray(b, dtype=np.float32)
    mask = np.asarray(mask)

    xp = np.zeros((C, H + 2, W + 2), dtype=np.float32)
    xp[:, 1:-1, 1:-1] = x[0]
    # phase split: xd[c, j, r, t] = xp[c, r, 4t + j],  j = 0..5, t = 0..47
    xd = np.empty((C, P, H + 2, T), dtype=np.float32)
    for j in range(P):
        xd[:, j] = xp[:, :, j:j + 4 * (T - 1) + 1:4]
    xd = xd.astype(ml_dtypes.bfloat16)

    _, G, _ = _wino_mats()
    # U[p, f, c, kh] = sum_j G[p, j] * w[f, c, kh, j]
    u = np.einsum("pj,fckj->pfck", G, w.astype(np.float64)).astype(np.float32)
    # -> [c_local, fm, p, cc, kh, f_local]
    u = u.reshape(P, FM, 128, CC, 128, KH)
    u = u.transpose(4, 1, 0, 3, 5, 2)
    u = np.ascontiguousarray(u).astype(ml_dtypes.bfloat16)

    b_re = np.ascontiguousarray(b.reshape(FM, 128).T)

    # mask k-split: mk[k, h, t] = mask[h, 4t + k]
    mk = np.ascontiguousarray(
        mask.reshape(H, T, 4).transpose(2, 0, 1)).astype(ml_dtypes.bfloat16)
    in_maps = []
    for k in range(N_CORES):
        # [cc, 128, P, XR, T] slab for this core
        xs = xd[:, :, HC * k:HC * k + XR, :].reshape(CC, 128, P, XR, T)
        # flat segments: per (chunk, cc-pair): [128, 2, P, nr, T]
        segs = []
        for ci, (r0, nr) in enumerate(CHUNKS):
            for pr in range(2):
                seg = xs[2 * pr:2 * pr + 2, :, :, r0:r0 + nr, :]
                segs.append(np.ascontiguousarray(
                    seg.transpose(1, 0, 2, 3, 4)).reshape(128, -1))
        xflat = np.concatenate(segs, axis=1)
        assert xflat.shape == (128, XTOT)
        ms = mk[:, HC * k:HC * k + HC][None]                # [1, 4, 24, 48]
        in_maps.append({
            "x_sh": np.ascontiguousarray(xflat),
            "u_sh": u,
            "mk_sh": np.ascontiguousarray(
                np.broadcast_to(ms, (128, 4, HC, T))),
            "b_sh": b_re,
        })
    return in_maps


def _unpack(results):
    slabs = []
    for k in range(N_CORES):
        ys = results[k]["y_sh"]                 # [FM, NG, 128, 4, RG, T] bf16
        ys = ys.transpose(0, 2, 1, 4, 5, 3)     # [FM, 128, NG, RG, T, 4]
        slabs.append(ys.reshape(F, HC, W))
    out = np.concatenate(slabs, axis=1)         # [512, 192, 192]
    return out[None].astype(np.float32)


def _run(inputs, **run_kwargs):
    from concourse.bass_utils import run_bass_kernel_spmd

    if "nc" not in _CACHE:
        _CACHE["nc"] = _build()
    nc = _CACHE["nc"]
    in_maps = _pack(inputs["x"], inputs["w"], inputs["b"], inputs["mask"])
    res = run_bass_kernel_spmd(nc, in_maps, core_ids=list(range(N_CORES)), **run_kwargs)
    return _unpack(res.results), res


def kernel(**inputs):
    out, _ = _run(inputs)
    return out
